# revision 15
# baseline (speedup 1.0000x reference)
"""Trainium2 Bass kernel: 16-head MHA with RoPE (B=4, N=2048, D=1024).

Sharding (8 cores): core c -> (batch b = c//2, head-group g = c%2 of 8 heads).
Each core computes its 8 heads' attention for one batch and a partial
projection output; the host sums the two partials per batch and adds b_proj.

v2: all matmul operands bf16 (FWL-eligible, halved DMA); scores emitted as
bf16 PSUM tiles at N=1024; softmax exp split three ways across engines
(ScalarE true exp / DVE + GpSimd Schraudolph fastexp writing int16 bits
read back as bf16); denominators obtained by replicating a 64-wide ones
block in the AV stationary ([v_even|ones|v_odd] per head pair) so both
heads' denominators land on 64 partitions each (reciprocal_approx_fast +
one multiply, no partition broadcast); projection evacuated by ScalarE
copies and DMA'd out as bf16.
"""

import numpy as np

EMBED = 1024
NHEAD = 16
HD = 64
SCALE = HD ** -0.5
B = 4
N = 2048
NCORES = 8

# Schraudolph fastexp: int16(x*SCALE*128*log2(e) + (16256 - 5.5)) bits = bf16(e^(x*SCALE))
FEXP_A = SCALE * 128.0 * 1.4426950408889634
FEXP_B = 16256.0 - 5.5

# exp-evacuation engine split per (kc, hi) index 0..31: counts per 32
# (GPSIMD cannot read PSUM, so the pool share must stay 0 for exp)
ACT_N, DVE_N, POOL_N = 22, 10, 0

_CACHE = {}


def _exp_engine_pattern():
    # weighted round-robin interleave of the three engines over 32 slots
    counts = {"act": ACT_N, "dve": DVE_N, "pool": POOL_N}
    acc = {k: 0.0 for k in counts}
    pat = []
    for _ in range(32):
        for k in counts:
            acc[k] += counts[k] / 32.0
        pick = max(acc, key=lambda k: acc[k])
        acc[pick] -= 1.0
        pat.append(pick)
    return pat


def _build_nc(niter=1, phases=("qkv", "attn", "proj")):
    import concourse.bacc as bacc
    import concourse.mybir as mybir
    from concourse.tile import TileContext

    f32 = mybir.dt.float32
    bf16 = mybir.dt.bfloat16
    nc = bacc.Bacc(None, target_bir_lowering=False)

    xT = nc.dram_tensor("xT", [EMBED, N], bf16, kind="ExternalInput")
    wqk = nc.dram_tensor("wqk", [EMBED, 1024], bf16, kind="ExternalInput")
    wv = nc.dram_tensor("wv", [EMBED, 512], bf16, kind="ExternalInput")
    wp = nc.dram_tensor("wp", [512, 1024], bf16, kind="ExternalInput")
    bqk = nc.dram_tensor("bqk", [128, 8], f32, kind="ExternalInput")
    cos2 = nc.dram_tensor("cos2", [128, N], bf16, kind="ExternalInput")
    sinS = nc.dram_tensor("sinS", [128, N], bf16, kind="ExternalInput")
    bvo = nc.dram_tensor("bvo", [128, 512], f32, kind="ExternalInput")
    y = nc.dram_tensor("y", [N, 1024], bf16, kind="ExternalOutput")

    with TileContext(nc) as tc:
        for _ in range(niter):
            _emit_iter(nc, tc, mybir,
                       xT, wqk, wv, wp, bqk, cos2, sinS, bvo, y, phases)

    nc.finalize()
    return nc


def _emit_iter(nc, tc, mybir,
               xT, wqk, wv, wp, bqk, cos2, sinS, bvo, y,
               phases=("qkv", "attn", "proj")):
    f32 = mybir.dt.float32
    bf16 = mybir.dt.bfloat16
    i16 = mybir.dt.int16
    A = mybir.AluOpType
    Act = mybir.ActivationFunctionType
    VBLK = 192  # per-pair block in the packed V tile: [v_even|ones|v_odd]
    MASK = [(i + 16) % 32 for i in range(32)]  # rotate-half under interleaved layout

    exp_pat = _exp_engine_pattern()

    with tc.tile_pool(name="persist", bufs=1) as Pp:
        qk_t = [Pp.tile([128, N], bf16, tag=f"qk{i}", name=f"qk{i}") for i in range(8)]
        va_t = [Pp.tile([128, 4 * VBLK], bf16, tag=f"va{i}", name=f"va{i}")
                for i in range(16)]
        oT_t = [Pp.tile([128, N], bf16, tag=f"o{j}", name=f"o{j}") for j in range(4)]

        # ---------------- phase 1: QKV + RoPE ----------------
        with (
            tc.tile_pool(name="qkv", bufs=1) as Pq,
            tc.tile_pool(name="ps_qk", bufs=2, space="PSUM") as Sqk,
            tc.tile_pool(name="ps_v", bufs=2, space="PSUM") as Sv,
        ):
            c2t = Pq.tile([128, N], bf16, tag="cos2")
            sSt = Pq.tile([128, N], bf16, tag="sinS")
            bqt = Pq.tile([128, 8], f32, tag="bqk")
            bvt = Pq.tile([128, 512], f32, tag="bvo")
            def load_xt(half):
                hsl = slice(1024 * half, 1024 * half + 1024)
                tiles = []
                for dk in range(8):
                    t = Pq.tile([128, 1024], bf16, tag=f"xt{dk}", bufs=2,
                                name=f"xt{dk}_{half}")
                    nc.sync.dma_start(out=t[:], in_=xT[128 * dk:128 * dk + 128, hsl])
                    tiles.append(t)
                return tiles

            wvt = []
            wqt = []
            if "qkv" in phases:
                nc.sync.dma_start(out=bqt[:], in_=bqk[:, :])
                nc.sync.dma_start(out=bvt[:], in_=bvo[:, :])
                # warm the exp activation table before phase 2 needs it
                warm = Pq.tile([1, 8], f32, tag="warmexp")
                nc.scalar.activation(warm[:], bqt[0:1, :], Act.Exp)
                for dk in range(8):
                    w = Pq.tile([128, 512], bf16, tag=f"wv{dk}", name=f"wv{dk}")
                    nc.sync.dma_start(out=w[:], in_=wv[128 * dk:128 * dk + 128, :])
                    wvt.append(w)
                xt_next = load_xt(0)
                for dk in range(8):
                    wq = Pq.tile([128, 1024], bf16, tag=f"wq{dk}", name=f"wq{dk}")
                    nc.sync.dma_start(out=wq[:], in_=wqk[128 * dk:128 * dk + 128, :])
                    wqt.append(wq)
                nc.sync.dma_start(out=c2t[:], in_=cos2[:, :])
                nc.sync.dma_start(out=sSt[:], in_=sinS[:, :])

            for half in range(2 if "qkv" in phases else 0):
                hsl = slice(1024 * half, 1024 * half + 1024)
                xt = xt_next
                if half == 0:
                    xt_next = load_xt(1)

                for ct in range(8):
                    rt = 8 * half + ct
                    # V for this 128-token block
                    pv = Sv.tile([128, 512], f32, tag="v")
                    for dk in range(8):
                        nc.tensor.matmul(
                            pv[:],
                            lhsT=xt[dk][:, 128 * ct:128 * ct + 128],
                            rhs=wvt[dk][:],
                            start=(dk == 0), stop=(dk == 7))
                    # q^T/k^T col-tile ct for this half's 1024 tokens
                    pqk = Sqk.tile([128, 1024], f32, tag="qk")
                    for dk in range(8):
                        for qc in range(2):
                            nc.tensor.matmul(
                                pqk[:, 512 * qc:512 * qc + 512],
                                lhsT=wqt[dk][:, 128 * ct:128 * ct + 128],
                                rhs=xt[dk][:, 512 * qc:512 * qc + 512],
                                start=(dk == 0), stop=(dk == 7))
                    dst = qk_t[ct][:, hsl]
                    # cos part: dst = (pqk + b) * cos
                    nc.vector.scalar_tensor_tensor(
                        out=dst, in0=pqk[:], scalar=bqt[:, ct:ct + 1],
                        in1=c2t[:, hsl], op0=A.add, op1=A.mult)
                    # sin part: premultiply by shuffled sign*sin, then the
                    # quadrant swap (rotate-half under the interleaved head
                    # layout) on the DVE shuffle unit, accumulate on Pool
                    u = Pq.tile([128, 1024], bf16, tag="ropeu", bufs=2)
                    nc.vector.scalar_tensor_tensor(
                        out=u[:], in0=pqk[:], scalar=bqt[:, ct:ct + 1],
                        in1=sSt[:, hsl], op0=A.add, op1=A.mult)
                    scr = Pq.tile([128, 1024], bf16, tag="ropescr", bufs=2)
                    nc.vector.stream_shuffle(scr[:], u[:], MASK)
                    nc.gpsimd.tensor_add(dst, dst, scr[:])

                    # pack V + bias into [v_even | ones | v_odd] per pair
                    va = va_t[rt]
                    vav = va[:].rearrange("p (j s c) -> p j s c", j=4, s=3, c=64)
                    pvv = pv[:].rearrange("p (j s c) -> p j s c", j=4, s=2, c=64)
                    bvv = bvt[:].rearrange("p (j s c) -> p j s c", j=4, s=2, c=64)
                    nc.vector.tensor_add(vav[:, :, 0, :], pvv[:, :, 0, :],
                                         bvv[:, :, 0, :])
                    nc.vector.tensor_add(vav[:, :, 2, :], pvv[:, :, 1, :],
                                         bvv[:, :, 1, :])
                    nc.vector.memset(vav[:, :, 1, :], 1.0)

        # ---------------- phase 2: attention ----------------
        with tc.tile_pool(name="wp", bufs=1) as Pw:
          wpt = []
          for j in range(4 if "proj" in phases else 0):
              t = Pw.tile([128, 1024], bf16, tag=f"wp{j}", name=f"wp{j}")
              nc.sync.dma_start(out=t[:], in_=wp[128 * j:128 * j + 128, :])
              wpt.append(t)
          with (
            tc.tile_pool(name="attn", bufs=1) as Pa,
            tc.tile_pool(name="ps_s", bufs=2, space="PSUM") as Ss,
            tc.tile_pool(name="ps_av", bufs=2, space="PSUM") as Sav,
          ):
            for j in range(4 if "attn" in phases else 0):
                qT_, kT_ = qk_t[j], qk_t[4 + j]
                vlo = VBLK * j
                for qh in range(2):
                    qsl = slice(1024 * qh, 1024 * qh + 1024)
                    av = [Sav.tile([128, 1024], f32, tag="av", name=f"av{i}")
                          for i in range(2)]
                    pT = [[None] * 16, [None] * 16]
                    for kc in range(16):
                        ks = slice(128 * kc, 128 * kc + 128)
                        s2 = [Ss.tile([128, 1024], f32, tag="s", name=f"s{i}")
                              for i in range(2)]
                        for qc in range(2):
                            qq = slice(1024 * qh + 512 * qc, 1024 * qh + 512 * qc + 512)
                            nc.tensor.matmul(
                                s2[0][:, 512 * qc:512 * qc + 512],
                                lhsT=kT_[0:64, ks], rhs=qT_[0:64, qq],
                                tile_position=(0, 0), start=True, stop=True)
                            nc.tensor.matmul(
                                s2[1][:, 512 * qc:512 * qc + 512],
                                lhsT=kT_[64:128, ks], rhs=qT_[64:128, qq],
                                tile_position=(64, 0), start=True, stop=True)
                        for hi in range(2):
                            p = Pa.tile([128, 1024], bf16, tag=f"p{hi}", bufs=4,
                                        name=f"p{hi}_{kc}")
                            pT[hi][kc] = p
                            eng = exp_pat[2 * kc + hi]
                            if eng == "act":
                                nc.scalar.activation(
                                    p[:], s2[hi][:], Act.Exp, scale=SCALE)
                            elif eng == "dve":
                                nc.vector.tensor_scalar(
                                    out=p[:].bitcast(i16), in0=s2[hi][:],
                                    scalar1=FEXP_A, scalar2=FEXP_B,
                                    op0=A.mult, op1=A.add)
                            else:
                                nc.gpsimd.tensor_scalar(
                                    out=p[:].bitcast(i16), in0=s2[hi][:],
                                    scalar1=FEXP_A, scalar2=FEXP_B,
                                    op0=A.mult, op1=A.add)
                        if kc > 0:
                            _emit_av(nc, va_t, pT, av, vlo, kc - 1)
                    _emit_av(nc, va_t, pT, av, vlo, 15)
                    # normalize + evacuate. DVE cannot shift partition bases
                    # on HW (only single-row reads work), so: row-read
                    # reciprocal, gpsimd broadcast, base-consistent multiply.
                    denA = Pa.tile([1, 1024], f32, tag="denA", bufs=2)
                    nc.scalar.copy(denA[:], av[0][64:65, :])
                    recA = Pa.tile([1, 1024], f32, tag="recA", bufs=2)
                    recB = Pa.tile([1, 1024], f32, tag="recB", bufs=2)
                    nc.vector.reciprocal_approx_fast(recA[:], denA[:])
                    nc.vector.reciprocal_approx_fast(recB[:], av[1][0:1, :])
                    rbcA = Pa.tile([128, 1024], f32, tag="rbcA", bufs=2)
                    rbcB = Pa.tile([128, 1024], f32, tag="rbcB", bufs=2)
                    nc.gpsimd.partition_broadcast(rbcA[:, :], recA[0:1, :])
                    nc.gpsimd.partition_broadcast(rbcB[:, :], recB[0:1, :])
                    nc.vector.tensor_mul(oT_t[j][0:64, qsl], av[0][0:64, :],
                                         rbcA[0:64, :])
                    nc.vector.tensor_mul(oT_t[j][64:128, qsl], av[1][64:128, :],
                                         rbcB[64:128, :])

        # ---------------- phase 3: projection ----------------
          with (
            tc.tile_pool(name="proj", bufs=1) as Pj,
            tc.tile_pool(name="ps_y", bufs=3, space="PSUM") as Sy,
          ):
            for rt in range(16 if "proj" in phases else 0):
                py = Sy.tile([128, 1024], f32, tag="y")
                for j in range(4):
                    for yc in range(2):
                        nc.tensor.matmul(
                            py[:, 512 * yc:512 * yc + 512],
                            lhsT=oT_t[j][:, 128 * rt:128 * rt + 128],
                            rhs=wpt[j][:, 512 * yc:512 * yc + 512],
                            start=(j == 0), stop=(j == 3))
                ysb = Pj.tile([128, 1024], bf16, tag="ysb", bufs=4)
                nc.scalar.copy(ysb[:], py[:])
                nc.sync.dma_start(out=y[128 * rt:128 * rt + 128, :], in_=ysb[:])


def _emit_av(nc, va_t, pT, av, vlo, kc):
    # av0: [O_even(64) ; denomA(64)] <- [v_even|ones].T @ P_A
    # av1: [denomB(64) ; O_odd(64)]  <- [ones|v_odd].T @ P_B
    for hi in range(2):
        lo = vlo + (64 if hi else 0)
        for qc in range(2):
            nc.tensor.matmul(
                av[hi][:, 512 * qc:512 * qc + 512],
                lhsT=va_t[kc][:, lo:lo + 128],
                rhs=pT[hi][kc][:, 512 * qc:512 * qc + 512],
                start=(kc == 0), stop=(kc == 15))


def _shard_inputs(x, rope_cos, rope_sin, W_qkv, b_qkv, W_proj):
    import ml_dtypes
    bf16 = ml_dtypes.bfloat16

    # Interleaved head-dim layout so rotate-half becomes a 32-lane quadrant
    # swap: position p (0..63) holds dim PERM[p]; SRC is the swap (involution)
    PERM = np.array(list(range(0, 16)) + list(range(32, 48))
                    + list(range(16, 32)) + list(range(48, 64)))
    SRC64 = np.array([(p % 32 + 16) % 32 + 32 * (p // 32) for p in range(64)])
    # cos2[p, t] = cos[t, dim(p)]
    cos1 = rope_cos.T[PERM]                       # [64, N]
    cos2 = np.ascontiguousarray(
        np.concatenate([cos1, cos1], 0)).astype(bf16)
    # sinS[p, t] = sign(src(p)) * sin[t, dim(src(p))]; sign(q) = -1 iff dim(q)<32
    dsrc = PERM[SRC64]
    sign = np.where(dsrc < 32, -1.0, 1.0)[:, None]
    sin1 = rope_sin.T[dsrc] * sign                # [64, N]
    sinS = np.ascontiguousarray(np.concatenate([sin1, sin1], 0)).astype(bf16)

    perm_head = np.concatenate([64 * h + PERM for h in range(8)])  # 512 cols

    per_g = []
    for g in range(2):
        o = 512 * g
        wq_g = W_qkv[:, o:o + 512][:, perm_head]
        wk_g = W_qkv[:, 1024 + o:1024 + o + 512][:, perm_head]
        wqk = np.ascontiguousarray(np.concatenate([wq_g, wk_g], 1)).astype(bf16)
        wv = np.ascontiguousarray(W_qkv[:, 2048 + o:2048 + o + 512]).astype(bf16)
        wp = np.ascontiguousarray(W_proj[o:o + 512, :]).astype(bf16)
        bqk_cat = np.concatenate([b_qkv[o:o + 512][perm_head],
                                  b_qkv[1024 + o:1024 + o + 512][perm_head]])
        bqk = np.ascontiguousarray(bqk_cat.reshape(8, 128).T, dtype=np.float32)
        bvo = np.ascontiguousarray(
            np.broadcast_to(b_qkv[2048 + o:2048 + o + 512], (128, 512)),
            dtype=np.float32)
        per_g.append(dict(wqk=wqk, wv=wv, wp=wp, bqk=bqk, bvo=bvo))

    xTs = [np.ascontiguousarray(x[b].T).astype(bf16) for b in range(B)]
    in_maps = []
    for c in range(NCORES):
        b, g = c // 2, c % 2
        m = dict(per_g[g])
        m["xT"] = xTs[b]
        m["cos2"] = cos2
        m["sinS"] = sinS
        in_maps.append(m)
    return in_maps


def _get_runner(niter=1):
    key = ("runner", niter)
    if key in _CACHE:
        return _CACHE[key]
    import jax
    from jax.sharding import Mesh, PartitionSpec
    from jax.experimental.shard_map import shard_map
    from concourse import bass2jax

    nc = _build_nc(niter)
    bass2jax.install_neuronx_cc_hook()

    import concourse.mybir as mybir
    partition_name = nc.partition_id_tensor.name if nc.partition_id_tensor else None
    in_names, out_names, out_avals, zero_outs = [], [], [], []
    for alloc in nc.m.functions[0].allocations:
        if not isinstance(alloc, mybir.MemoryLocationSet):
            continue
        name = alloc.memorylocations[0].name
        if alloc.kind == "ExternalInput":
            if name != partition_name:
                in_names.append(name)
        elif alloc.kind == "ExternalOutput":
            shape = tuple(alloc.tensor_shape)
            np_dtype = mybir.dt.np(alloc.dtype)
            out_names.append(name)
            out_avals.append(jax.core.ShapedArray(shape, np_dtype))
            zero_outs.append(np.zeros(shape, np_dtype))

    n_params = len(in_names)
    n_outs = len(out_names)
    all_in_names = list(in_names) + list(out_names)
    if partition_name is not None:
        all_in_names.append(partition_name)
    donate = tuple(range(n_params, n_params + n_outs))

    def _body(*args):
        operands = list(args)
        if partition_name is not None:
            operands.append(bass2jax.partition_id_tensor())
        outs = bass2jax._bass_exec_p.bind(
            *operands,
            out_avals=tuple(out_avals),
            in_names=tuple(all_in_names),
            out_names=tuple(out_names),
            lowering_input_output_aliases=(),
            sim_require_finite=True,
            sim_require_nnan=True,
            nc=nc,
        )
        return tuple(outs)

    devices = jax.devices()[:NCORES]
    mesh = Mesh(np.asarray(devices), ("core",))
    in_specs = (PartitionSpec("core"),) * (n_params + n_outs)
    out_specs = (PartitionSpec("core"),) * n_outs
    sharded = jax.jit(
        shard_map(_body, mesh=mesh, in_specs=in_specs, out_specs=out_specs,
                  check_rep=False),
        donate_argnums=donate, keep_unused=True)

    runner = dict(sharded=sharded, in_names=in_names, out_names=out_names,
                  out_avals=out_avals, zero_outs=zero_outs, nc=nc)
    _CACHE[key] = runner
    return runner


def _run_spmd(in_maps, niter=1):
    r = _get_runner(niter)
    concat_in = [
        np.concatenate([np.asarray(in_maps[c][name]) for c in range(NCORES)], axis=0)
        for name in r["in_names"]
    ]
    concat_zeros = [
        np.zeros((NCORES * z.shape[0], *z.shape[1:]), z.dtype) for z in r["zero_outs"]
    ]
    out_arrs = r["sharded"](*concat_in, *concat_zeros)
    outs = []
    for c in range(NCORES):
        m = {}
        for i, name in enumerate(r["out_names"]):
            shape = r["out_avals"][i].shape
            m[name] = np.asarray(out_arrs[i]).reshape(NCORES, *shape)[c]
        outs.append(m)
    return outs


def kernel(**inputs):
    x = np.asarray(inputs["x"], np.float32)
    rope_cos = np.asarray(inputs["rope_cos"], np.float32)
    rope_sin = np.asarray(inputs["rope_sin"], np.float32)
    W_qkv = np.asarray(inputs["W_qkv"], np.float32)
    b_qkv = np.asarray(inputs["b_qkv"], np.float32)
    W_proj = np.asarray(inputs["W_proj"], np.float32)
    b_proj = np.asarray(inputs["b_proj"], np.float32)

    in_maps = _shard_inputs(x, rope_cos, rope_sin, W_qkv, b_qkv, W_proj)
    outs = _run_spmd(in_maps)
    out = np.empty((B, N, EMBED), np.float32)
    for b in range(B):
        out[b] = (outs[2 * b]["y"].astype(np.float32)
                  + outs[2 * b + 1]["y"].astype(np.float32) + b_proj)
    return out


# revision 18
# speedup vs baseline: 1.5545x; 1.5545x over previous
"""Trainium2 Bass kernel: 16-head MHA with RoPE (B=4, N=2048, D=1024).

Sharding (8 cores): core c -> (batch b = c//2, head-group g = c%2 of 8 heads).
Each core computes its 8 heads' attention for one batch and a partial
projection output; the host sums the two partials per batch and adds b_proj.

v2: all matmul operands bf16 (FWL-eligible, halved DMA); scores emitted as
bf16 PSUM tiles at N=1024; softmax exp split three ways across engines
(ScalarE true exp / DVE + GpSimd Schraudolph fastexp writing int16 bits
read back as bf16); denominators obtained by replicating a 64-wide ones
block in the AV stationary ([v_even|ones|v_odd] per head pair) so both
heads' denominators land on 64 partitions each (reciprocal_approx_fast +
one multiply, no partition broadcast); projection evacuated by ScalarE
copies and DMA'd out as bf16.
"""

import numpy as np

EMBED = 1024
NHEAD = 16
HD = 64
SCALE = HD ** -0.5
B = 4
N = 2048
NCORES = 8

# Schraudolph fastexp: int16(x*SCALE*128*log2(e) + (16256 - 5.5)) bits = bf16(e^(x*SCALE))
FEXP_A = SCALE * 128.0 * 1.4426950408889634
FEXP_B = 16256.0 - 5.5

# exp-evacuation engine split per (kc, hi) index 0..31: counts per 32
# (GPSIMD cannot read PSUM, so the pool share must stay 0 for exp)
ACT_N, DVE_N, POOL_N = 22, 10, 0

_CACHE = {}


def _exp_engine_pattern():
    # weighted round-robin interleave of the three engines over 32 slots
    counts = {"act": ACT_N, "dve": DVE_N, "pool": POOL_N}
    acc = {k: 0.0 for k in counts}
    pat = []
    for _ in range(32):
        for k in counts:
            acc[k] += counts[k] / 32.0
        pick = max(acc, key=lambda k: acc[k])
        acc[pick] -= 1.0
        pat.append(pick)
    return pat


def _build_nc(niter=1, phases=("qkv", "attn", "proj")):
    import concourse.bacc as bacc
    import concourse.mybir as mybir
    from concourse.tile import TileContext

    f32 = mybir.dt.float32
    bf16 = mybir.dt.bfloat16
    nc = bacc.Bacc(None, target_bir_lowering=False)

    xT = nc.dram_tensor("xT", [EMBED, N], bf16, kind="ExternalInput")
    wqk = nc.dram_tensor("wqk", [EMBED, 1024], bf16, kind="ExternalInput")
    wv = nc.dram_tensor("wv", [EMBED, 512], bf16, kind="ExternalInput")
    wp = nc.dram_tensor("wp", [512, 1024], bf16, kind="ExternalInput")
    bqk = nc.dram_tensor("bqk", [128, 8], f32, kind="ExternalInput")
    cos2 = nc.dram_tensor("cos2", [128, N], bf16, kind="ExternalInput")
    sinS = nc.dram_tensor("sinS", [128, N], bf16, kind="ExternalInput")
    bvo = nc.dram_tensor("bvo", [128, 512], f32, kind="ExternalInput")
    y = nc.dram_tensor("y", [N, 1024], bf16, kind="ExternalOutput")

    with TileContext(nc) as tc:
        with tc.tile_pool(name="persist", bufs=1) as Pp:
            for _ in range(niter):
                _emit_iter(nc, tc, mybir, Pp,
                           xT, wqk, wv, wp, bqk, cos2, sinS, bvo, y, phases)

    nc.finalize()
    return nc


def _emit_iter(nc, tc, mybir, Pp,
               xT, wqk, wv, wp, bqk, cos2, sinS, bvo, y,
               phases=("qkv", "attn", "proj")):
    f32 = mybir.dt.float32
    bf16 = mybir.dt.bfloat16
    i16 = mybir.dt.int16
    A = mybir.AluOpType
    Act = mybir.ActivationFunctionType
    VBLK = 192  # per-pair block in the packed V tile: [v_even|ones|v_odd]
    MASK = [(i + 16) % 32 for i in range(32)]  # rotate-half under interleaved layout

    exp_pat = _exp_engine_pattern()

    if True:
        # qk/va double-buffered across loop iterations (the shared persist
        # pool rotates tag slots) so iter N+1's QKV writes fresh tiles while
        # iter N's attention still reads its own
        qk_t = [Pp.tile([128, N], bf16, tag=f"qk{i}", name=f"qk{i}", bufs=2)
                for i in range(8)]
        va_t = [Pp.tile([128, 4 * VBLK], bf16, tag=f"va{i}", name=f"va{i}",
                        bufs=2)
                for i in range(16)]
        oT_t = [Pp.tile([128, N], bf16, tag=f"o{j}", name=f"o{j}") for j in range(4)]

        # ---------------- phase 1: QKV + RoPE ----------------
        with (
            tc.tile_pool(name="qkv", bufs=1) as Pq,
            tc.tile_pool(name="ps_qk", bufs=2, space="PSUM") as Sqk,
            tc.tile_pool(name="ps_v", bufs=2, space="PSUM") as Sv,
        ):
            c2t = Pq.tile([128, N], bf16, tag="cos2")
            sSt = Pq.tile([128, N], bf16, tag="sinS")
            bqt = Pq.tile([128, 8], f32, tag="bqk")
            bvt = Pq.tile([128, 512], f32, tag="bvo")
            def load_xt(half):
                hsl = slice(1024 * half, 1024 * half + 1024)
                tiles = []
                for dk in range(8):
                    t = Pq.tile([128, 1024], bf16, tag=f"xt{dk}", bufs=2,
                                name=f"xt{dk}_{half}")
                    nc.sync.dma_start(out=t[:], in_=xT[128 * dk:128 * dk + 128, hsl])
                    tiles.append(t)
                return tiles

            wvt = []
            wqt = []
            if "qkv" in phases:
                nc.sync.dma_start(out=bqt[:], in_=bqk[:, :])
                nc.sync.dma_start(out=bvt[:], in_=bvo[:, :])
                # warm the exp activation table before phase 2 needs it
                warm = Pq.tile([1, 8], f32, tag="warmexp")
                nc.scalar.activation(warm[:], bqt[0:1, :], Act.Exp)
                for dk in range(8):
                    w = Pq.tile([128, 512], bf16, tag=f"wv{dk}", name=f"wv{dk}")
                    nc.sync.dma_start(out=w[:], in_=wv[128 * dk:128 * dk + 128, :])
                    wvt.append(w)
                xt_next = load_xt(0)
                for dk in range(8):
                    wq = Pq.tile([128, 1024], bf16, tag=f"wq{dk}", name=f"wq{dk}")
                    nc.sync.dma_start(out=wq[:], in_=wqk[128 * dk:128 * dk + 128, :])
                    wqt.append(wq)
                nc.sync.dma_start(out=c2t[:], in_=cos2[:, :])
                nc.sync.dma_start(out=sSt[:], in_=sinS[:, :])

            for half in range(2 if "qkv" in phases else 0):
                hsl = slice(1024 * half, 1024 * half + 1024)
                xt = xt_next
                if half == 0:
                    xt_next = load_xt(1)

                for ct in (0, 4, 1, 5, 2, 6, 3, 7):
                    rt = 8 * half + ct
                    # V for this 128-token block
                    pv = Sv.tile([128, 512], f32, tag="v")
                    for dk in range(8):
                        nc.tensor.matmul(
                            pv[:],
                            lhsT=xt[dk][:, 128 * ct:128 * ct + 128],
                            rhs=wvt[dk][:],
                            start=(dk == 0), stop=(dk == 7))
                    # q^T/k^T col-tile ct for this half's 1024 tokens
                    pqk = Sqk.tile([128, 1024], f32, tag="qk")
                    for dk in range(8):
                        for qc in range(2):
                            nc.tensor.matmul(
                                pqk[:, 512 * qc:512 * qc + 512],
                                lhsT=wqt[dk][:, 128 * ct:128 * ct + 128],
                                rhs=xt[dk][:, 512 * qc:512 * qc + 512],
                                start=(dk == 0), stop=(dk == 7))
                    dst = qk_t[ct][:, hsl]
                    # cos part: dst = (pqk + b) * cos
                    nc.vector.scalar_tensor_tensor(
                        out=dst, in0=pqk[:], scalar=bqt[:, ct:ct + 1],
                        in1=c2t[:, hsl], op0=A.add, op1=A.mult)
                    # sin part: premultiply by shuffled sign*sin, then the
                    # quadrant swap (rotate-half under the interleaved head
                    # layout) on the DVE shuffle unit, accumulate on Pool
                    u = Pq.tile([128, 1024], bf16, tag="ropeu", bufs=2)
                    nc.vector.scalar_tensor_tensor(
                        out=u[:], in0=pqk[:], scalar=bqt[:, ct:ct + 1],
                        in1=sSt[:, hsl], op0=A.add, op1=A.mult)
                    scr = Pq.tile([128, 1024], bf16, tag="ropescr", bufs=2)
                    nc.vector.stream_shuffle(scr[:], u[:], MASK)
                    nc.gpsimd.tensor_add(dst, dst, scr[:])

                    # pack V + bias into [v_even | ones | v_odd] per pair
                    va = va_t[rt]
                    vav = va[:].rearrange("p (j s c) -> p j s c", j=4, s=3, c=64)
                    pvv = pv[:].rearrange("p (j s c) -> p j s c", j=4, s=2, c=64)
                    bvv = bvt[:].rearrange("p (j s c) -> p j s c", j=4, s=2, c=64)
                    nc.vector.tensor_add(vav[:, :, 0, :], pvv[:, :, 0, :],
                                         bvv[:, :, 0, :])
                    nc.vector.tensor_add(vav[:, :, 2, :], pvv[:, :, 1, :],
                                         bvv[:, :, 1, :])
                    nc.vector.memset(vav[:, :, 1, :], 1.0)

        # ---------------- phase 2: attention ----------------
        with tc.tile_pool(name="wp", bufs=1) as Pw:
          wpt = []
          for j in range(4 if "proj" in phases else 0):
              t = Pw.tile([128, 1024], bf16, tag=f"wp{j}", name=f"wp{j}")
              nc.sync.dma_start(out=t[:], in_=wp[128 * j:128 * j + 128, :])
              wpt.append(t)
          with (
            tc.tile_pool(name="attn", bufs=1) as Pa,
            tc.tile_pool(name="ps_s", bufs=2, space="PSUM") as Ss,
            tc.tile_pool(name="ps_av", bufs=2, space="PSUM") as Sav,
          ):
            for j in range(4 if "attn" in phases else 0):
                qT_, kT_ = qk_t[j], qk_t[4 + j]
                vlo = VBLK * j
                for qh in range(2):
                    qsl = slice(1024 * qh, 1024 * qh + 1024)
                    av = [Sav.tile([128, 1024], f32, tag="av", name=f"av{i}")
                          for i in range(2)]
                    pT = [[None] * 16, [None] * 16]
                    for kc in range(16):
                        ks = slice(128 * kc, 128 * kc + 128)
                        s2 = [Ss.tile([128, 1024], f32, tag="s", name=f"s{i}")
                              for i in range(2)]
                        for qc in range(2):
                            qq = slice(1024 * qh + 512 * qc, 1024 * qh + 512 * qc + 512)
                            nc.tensor.matmul(
                                s2[0][:, 512 * qc:512 * qc + 512],
                                lhsT=kT_[0:64, ks], rhs=qT_[0:64, qq],
                                tile_position=(0, 0), start=True, stop=True)
                            nc.tensor.matmul(
                                s2[1][:, 512 * qc:512 * qc + 512],
                                lhsT=kT_[64:128, ks], rhs=qT_[64:128, qq],
                                tile_position=(64, 0), start=True, stop=True)
                        for hi in range(2):
                            p = Pa.tile([128, 1024], bf16, tag=f"p{hi}", bufs=4,
                                        name=f"p{hi}_{kc}")
                            pT[hi][kc] = p
                            eng = exp_pat[2 * kc + hi]
                            if eng == "act":
                                nc.scalar.activation(
                                    p[:], s2[hi][:], Act.Exp, scale=SCALE)
                            elif eng == "dve":
                                nc.vector.tensor_scalar(
                                    out=p[:].bitcast(i16), in0=s2[hi][:],
                                    scalar1=FEXP_A, scalar2=FEXP_B,
                                    op0=A.mult, op1=A.add)
                            else:
                                nc.gpsimd.tensor_scalar(
                                    out=p[:].bitcast(i16), in0=s2[hi][:],
                                    scalar1=FEXP_A, scalar2=FEXP_B,
                                    op0=A.mult, op1=A.add)
                        if kc > 0:
                            _emit_av(nc, va_t, pT, av, vlo, kc - 1)
                    _emit_av(nc, va_t, pT, av, vlo, 15)
                    # normalize + evacuate. DVE cannot shift partition bases
                    # on HW (only single-row reads work), so: row-read
                    # reciprocal, gpsimd broadcast, base-consistent multiply.
                    denA = Pa.tile([1, 1024], f32, tag="denA", bufs=2)
                    nc.scalar.copy(denA[:], av[0][64:65, :])
                    recA = Pa.tile([1, 1024], f32, tag="recA", bufs=2)
                    recB = Pa.tile([1, 1024], f32, tag="recB", bufs=2)
                    nc.vector.reciprocal_approx_fast(recA[:], denA[:])
                    nc.vector.reciprocal_approx_fast(recB[:], av[1][0:1, :])
                    rbcA = Pa.tile([128, 1024], f32, tag="rbcA", bufs=2)
                    rbcB = Pa.tile([128, 1024], f32, tag="rbcB", bufs=2)
                    nc.gpsimd.partition_broadcast(rbcA[:, :], recA[0:1, :])
                    nc.gpsimd.partition_broadcast(rbcB[:, :], recB[0:1, :])
                    nc.vector.tensor_mul(oT_t[j][0:64, qsl], av[0][0:64, :],
                                         rbcA[0:64, :])
                    nc.vector.tensor_mul(oT_t[j][64:128, qsl], av[1][64:128, :],
                                         rbcB[64:128, :])

        # ---------------- phase 3: projection ----------------
          with (
            tc.tile_pool(name="proj", bufs=1) as Pj,
            tc.tile_pool(name="ps_y", bufs=3, space="PSUM") as Sy,
          ):
            for rt in range(16 if "proj" in phases else 0):
                py = Sy.tile([128, 1024], f32, tag="y")
                for j in range(4):
                    for yc in range(2):
                        nc.tensor.matmul(
                            py[:, 512 * yc:512 * yc + 512],
                            lhsT=oT_t[j][:, 128 * rt:128 * rt + 128],
                            rhs=wpt[j][:, 512 * yc:512 * yc + 512],
                            start=(j == 0), stop=(j == 3))
                ysb = Pj.tile([128, 1024], bf16, tag="ysb", bufs=4)
                nc.scalar.copy(ysb[:], py[:])
                nc.sync.dma_start(out=y[128 * rt:128 * rt + 128, :], in_=ysb[:])


def _emit_av(nc, va_t, pT, av, vlo, kc):
    # av0: [O_even(64) ; denomA(64)] <- [v_even|ones].T @ P_A
    # av1: [denomB(64) ; O_odd(64)]  <- [ones|v_odd].T @ P_B
    for hi in range(2):
        lo = vlo + (64 if hi else 0)
        for qc in range(2):
            nc.tensor.matmul(
                av[hi][:, 512 * qc:512 * qc + 512],
                lhsT=va_t[kc][:, lo:lo + 128],
                rhs=pT[hi][kc][:, 512 * qc:512 * qc + 512],
                start=(kc == 0), stop=(kc == 15))


def _shard_inputs(x, rope_cos, rope_sin, W_qkv, b_qkv, W_proj):
    import ml_dtypes
    bf16 = ml_dtypes.bfloat16

    # Interleaved head-dim layout so rotate-half becomes a 32-lane quadrant
    # swap: position p (0..63) holds dim PERM[p]; SRC is the swap (involution)
    PERM = np.array(list(range(0, 16)) + list(range(32, 48))
                    + list(range(16, 32)) + list(range(48, 64)))
    SRC64 = np.array([(p % 32 + 16) % 32 + 32 * (p // 32) for p in range(64)])
    # cos2[p, t] = cos[t, dim(p)]
    cos1 = rope_cos.T[PERM]                       # [64, N]
    cos2 = np.ascontiguousarray(
        np.concatenate([cos1, cos1], 0)).astype(bf16)
    # sinS[p, t] = sign(src(p)) * sin[t, dim(src(p))]; sign(q) = -1 iff dim(q)<32
    dsrc = PERM[SRC64]
    sign = np.where(dsrc < 32, -1.0, 1.0)[:, None]
    sin1 = rope_sin.T[dsrc] * sign                # [64, N]
    sinS = np.ascontiguousarray(np.concatenate([sin1, sin1], 0)).astype(bf16)

    perm_head = np.concatenate([64 * h + PERM for h in range(8)])  # 512 cols

    per_g = []
    for g in range(2):
        o = 512 * g
        wq_g = W_qkv[:, o:o + 512][:, perm_head]
        wk_g = W_qkv[:, 1024 + o:1024 + o + 512][:, perm_head]
        wqk = np.ascontiguousarray(np.concatenate([wq_g, wk_g], 1)).astype(bf16)
        wv = np.ascontiguousarray(W_qkv[:, 2048 + o:2048 + o + 512]).astype(bf16)
        wp = np.ascontiguousarray(W_proj[o:o + 512, :]).astype(bf16)
        bqk_cat = np.concatenate([b_qkv[o:o + 512][perm_head],
                                  b_qkv[1024 + o:1024 + o + 512][perm_head]])
        bqk = np.ascontiguousarray(bqk_cat.reshape(8, 128).T, dtype=np.float32)
        bvo = np.ascontiguousarray(
            np.broadcast_to(b_qkv[2048 + o:2048 + o + 512], (128, 512)),
            dtype=np.float32)
        per_g.append(dict(wqk=wqk, wv=wv, wp=wp, bqk=bqk, bvo=bvo))

    xTs = [np.ascontiguousarray(x[b].T).astype(bf16) for b in range(B)]
    in_maps = []
    for c in range(NCORES):
        b, g = c // 2, c % 2
        m = dict(per_g[g])
        m["xT"] = xTs[b]
        m["cos2"] = cos2
        m["sinS"] = sinS
        in_maps.append(m)
    return in_maps


def _get_runner(niter=1):
    key = ("runner", niter)
    if key in _CACHE:
        return _CACHE[key]
    import jax
    from jax.sharding import Mesh, PartitionSpec
    from jax.experimental.shard_map import shard_map
    from concourse import bass2jax

    nc = _build_nc(niter)
    bass2jax.install_neuronx_cc_hook()

    import concourse.mybir as mybir
    partition_name = nc.partition_id_tensor.name if nc.partition_id_tensor else None
    in_names, out_names, out_avals, zero_outs = [], [], [], []
    for alloc in nc.m.functions[0].allocations:
        if not isinstance(alloc, mybir.MemoryLocationSet):
            continue
        name = alloc.memorylocations[0].name
        if alloc.kind == "ExternalInput":
            if name != partition_name:
                in_names.append(name)
        elif alloc.kind == "ExternalOutput":
            shape = tuple(alloc.tensor_shape)
            np_dtype = mybir.dt.np(alloc.dtype)
            out_names.append(name)
            out_avals.append(jax.core.ShapedArray(shape, np_dtype))
            zero_outs.append(np.zeros(shape, np_dtype))

    n_params = len(in_names)
    n_outs = len(out_names)
    all_in_names = list(in_names) + list(out_names)
    if partition_name is not None:
        all_in_names.append(partition_name)
    donate = tuple(range(n_params, n_params + n_outs))

    def _body(*args):
        operands = list(args)
        if partition_name is not None:
            operands.append(bass2jax.partition_id_tensor())
        outs = bass2jax._bass_exec_p.bind(
            *operands,
            out_avals=tuple(out_avals),
            in_names=tuple(all_in_names),
            out_names=tuple(out_names),
            lowering_input_output_aliases=(),
            sim_require_finite=True,
            sim_require_nnan=True,
            nc=nc,
        )
        return tuple(outs)

    devices = jax.devices()[:NCORES]
    mesh = Mesh(np.asarray(devices), ("core",))
    in_specs = (PartitionSpec("core"),) * (n_params + n_outs)
    out_specs = (PartitionSpec("core"),) * n_outs
    sharded = jax.jit(
        shard_map(_body, mesh=mesh, in_specs=in_specs, out_specs=out_specs,
                  check_rep=False),
        donate_argnums=donate, keep_unused=True)

    runner = dict(sharded=sharded, in_names=in_names, out_names=out_names,
                  out_avals=out_avals, zero_outs=zero_outs, nc=nc)
    _CACHE[key] = runner
    return runner


def _run_spmd(in_maps, niter=1):
    r = _get_runner(niter)
    concat_in = [
        np.concatenate([np.asarray(in_maps[c][name]) for c in range(NCORES)], axis=0)
        for name in r["in_names"]
    ]
    concat_zeros = [
        np.zeros((NCORES * z.shape[0], *z.shape[1:]), z.dtype) for z in r["zero_outs"]
    ]
    out_arrs = r["sharded"](*concat_in, *concat_zeros)
    outs = []
    for c in range(NCORES):
        m = {}
        for i, name in enumerate(r["out_names"]):
            shape = r["out_avals"][i].shape
            m[name] = np.asarray(out_arrs[i]).reshape(NCORES, *shape)[c]
        outs.append(m)
    return outs


def kernel(**inputs):
    x = np.asarray(inputs["x"], np.float32)
    rope_cos = np.asarray(inputs["rope_cos"], np.float32)
    rope_sin = np.asarray(inputs["rope_sin"], np.float32)
    W_qkv = np.asarray(inputs["W_qkv"], np.float32)
    b_qkv = np.asarray(inputs["b_qkv"], np.float32)
    W_proj = np.asarray(inputs["W_proj"], np.float32)
    b_proj = np.asarray(inputs["b_proj"], np.float32)

    in_maps = _shard_inputs(x, rope_cos, rope_sin, W_qkv, b_qkv, W_proj)
    outs = _run_spmd(in_maps)
    out = np.empty((B, N, EMBED), np.float32)
    for b in range(B):
        out[b] = (outs[2 * b]["y"].astype(np.float32)
                  + outs[2 * b + 1]["y"].astype(np.float32) + b_proj)
    return out
